# revision 16
# baseline (speedup 1.0000x reference)
"""Trainium2 Bass kernel for nn_AutoEncoder_48052094108202.

  h = x @ W1 + b1          # [B, H]
  y = h @ W2 + b2          # [B, D]
  out = segmented_softmax(y, segment_ids)   # softmax over contiguous
                                            # feature segments, per row

B=8192, D=4096, H=2048, S=512 segments. Data-parallel over B across 8
NeuronCores (1024 rows/core), weights replicated.

Per-core layout: everything runs transposed (features on SBUF partitions,
batch on the free axis) so no on-device transposes are needed — the host
pre-packs x^T (and un-transposes the output). The segmented softmax is done
entirely on the tensor engine with one-hot matmuls (exact — every product is
1.0 * x):
  seg_sums   s[seg, b] = C_g^T @ e     (C one-hot features->segments)
  recip      r = exp(-ln(s))           (two fast ACT passes, bf16 out)
  broadcast  d[feat, b] = C_g @ r      (one-hot rows)
  out        = e * d                   (DVE, bf16 out)
Matmuls run in bf16 (inputs/weights rounded on host), accumulation in fp32
PSUM. exp() on the ACT engine with the bias folded in. The single empty
segment row gets a 2^-60 weight on one feature in the host-built one-hot so
its sum stays a normal float (its reciprocal row is never read).

Segment groups are 128-segment blocks, split progressively finer over the
last k-tiles so each group's drain (reduce -> recip -> broadcast -> mul ->
store) starts as soon as its k-range completes instead of queueing behind
the final matmul.

Chunk 0's phase A opens with ONE k-outer pass over m-tiles 0-3 (4 PSUM
banks): during the startup DMA fill each arriving x k-tile feeds 4 matmuls
(~864ns of PE work), matching the DMA arrival rate, so the PE does useful
work instead of warm-up padding; the matching k-slices of the four w1 tiles
stream in k-major order. Everything else runs m-serial (a k-outer pass
holds all 4 banks to its end and then drains 4 ACTs serially, stalling the
next pass). Steady-state loads trigger from the sync engine (triggers can
carry ring-slot waits; on the ACT queue those block activations and stall
the PE) — only the wait-free startup w1 slices use the ACT engine's HWDGE
port, in parallel with sync's x stream. The batch shard is processed in 2
chunks of 512 columns; output is written bf16 and upcast on host.
"""

import os
import sys

import numpy as np

# ---------------------------------------------------------------- constants
B, D, H, S = 8192, 4096, 2048, 512
NCORES = 8
BS = B // NCORES  # 1024 batch rows per core
NB = 2  # chunks per core
BC = BS // NB  # 512 batch rows per chunk
KD = D // 128  # 32 k-tiles over D
KH = H // 128  # 16 k-tiles over H
MQ = 4  # phase-A m-tiles per k-outer pass

_WAIT_LIMIT = 1  # walrus CoreV3 accepts 1 sync-wait per instruction


def _import_concourse():
    try:
        import concourse  # noqa: F401
    except ImportError:
        for p in ("/opt/trn_rl_repo", "/root/.axon_site/_ro/trn_rl_repo"):
            if os.path.isdir(p) and p not in sys.path:
                sys.path.insert(0, p)
        import concourse  # noqa: F401


def _split_excess_waits(nc, limit=_WAIT_LIMIT):
    """walrus rejects instructions carrying more than one sync-wait; hoist
    extras onto preceding NOPs on the same engine (same semantics: blocking
    waits on one sequencer, order irrelevant)."""
    import bass_rust

    engines = nc.engines
    for fn in nc.m.functions:
        for bb in fn.blocks:
            insts = bb.instructions
            i = 0
            while i < len(insts):
                inst = insts[i]
                si = inst.sync_info
                waits = list(si.on_wait) if si and si.on_wait else []
                if len(waits) > limit:
                    overflow, keep = waits[:-limit], waits[-limit:]
                    si.on_wait = keep
                    pos = i
                    for j in range(0, len(overflow), limit):
                        nop = engines[inst.engine].nop(
                            nofuse=True, hint="wait_split"
                        ).ins
                        for b2 in fn.blocks:
                            lst = b2.instructions
                            if nop in lst:
                                lst.remove(nop)
                        nop.sync_info = bass_rust.SyncInfo(
                            on_wait=overflow[j : j + limit], on_update=[]
                        )
                        insts.insert(pos, nop)
                        pos += 1
                        i += 1
                i += 1


def _segment_plan(seg):
    """Static plan from the (sorted) segment ids.

    Groups are 128-aligned segment blocks, with the final block split at the
    first segment of the last k-tile so the kernel's drain group is tiny.

    Returns (bounds, kg_pairs, k_first, k_last, m_groups) where bounds[g] is
    the first segment id of group g (bounds has n_groups+1 entries).
    """
    seg = np.asarray(seg).astype(np.int64)
    assert seg.shape == (D,)
    bounds = [0, 128, 256, 384]
    # fine-grained groups over the last k-tiles: each completes (and its
    # broadcasts start) progressively instead of queueing one big group's
    # drain behind the final matmul
    for k in (26, 28, 30, 31):
        s_k = int(seg[k * 128])  # first segment of k-tile k
        if bounds[-1] < s_k < S:
            bounds.append(s_k)
    bounds.append(S)
    bounds = sorted(set(bounds))
    gof = np.searchsorted(np.asarray(bounds), np.arange(S), side="right") - 1

    kg_pairs = []
    for k in range(KD):
        gs = np.unique(gof[seg[k * 128 : (k + 1) * 128]])
        for g in gs:
            kg_pairs.append((k, int(g)))
    k_first = {}
    k_last = {}
    for k, g in kg_pairs:
        k_first.setdefault(g, k)
        k_last[g] = k
    m_groups = {}
    for k, g in kg_pairs:
        m_groups.setdefault(k, []).append(g)
    return bounds, kg_pairs, k_first, k_last, m_groups


def _build_program(seg):
    """Build the (SPMD, per-core) Bass program. Same program on all cores."""
    _import_concourse()
    import concourse.bass as bass
    import concourse.mybir as mybir
    from concourse import tile

    dt = mybir.dt
    AF = mybir.ActivationFunctionType

    bounds, kg_pairs, k_first, k_last, m_groups = _segment_plan(seg)
    NKG = len(kg_pairs)
    kg_index = {pair: i for i, pair in enumerate(kg_pairs)}

    nc = bass.Bass("TRN2", target_bir_lowering=False, debug=False)

    xtp = nc.dram_tensor("xtp", [NB, 128, KD, BC], dt.bfloat16, kind="ExternalInput")
    w1p = nc.dram_tensor("w1p", [KH, 128, KD, 128], dt.bfloat16, kind="ExternalInput")
    w2p = nc.dram_tensor("w2p", [KD, 128, KH, 128], dt.bfloat16, kind="ExternalInput")
    b1p = nc.dram_tensor("b1p", [128, KH], dt.float32, kind="ExternalInput")
    b2p = nc.dram_tensor("b2p", [128, KD], dt.float32, kind="ExternalInput")
    # one-hot tiles, partition-major so each loads as a single DMA
    cpp = nc.dram_tensor("cpp", [128, NKG, 128], dt.bfloat16, kind="ExternalInput")
    ctpp = nc.dram_tensor("ctpp", [128, NKG, 128], dt.bfloat16, kind="ExternalInput")
    outp = nc.dram_tensor("outp", [KD, 128, BS], dt.bfloat16, kind="ExternalOutput")

    with tile.TileContext(nc) as tc:
        with (
            tc.tile_pool(name="pbig", bufs=1) as pbig,
            tc.tile_pool(name="pw1", bufs=5) as pw1,
            tc.tile_pool(name="pw2", bufs=3) as pw2,
            tc.tile_pool(name="pev", bufs=3) as pev,
            tc.tile_pool(name="psmall", bufs=1) as psmall,
            tc.tile_pool(name="psum_mm", bufs=MQ, space="PSUM") as psum_mm,
            tc.tile_pool(name="psum_s", bufs=2, space="PSUM") as psum_s,
            tc.tile_pool(name="psum_d", bufs=2, space="PSUM") as psum_d,
        ):
            xts = {}

            def emit_xt_load(c, pairs=None):
                if c not in xts:
                    xts[c] = pbig.tile(
                        [128, KD, BC], dt.bfloat16, name=f"xt{c}", tag="xt", bufs=2
                    )
                xt = xts[c]
                if pairs is not None:
                    ks = [(k, 2) for k in pairs]
                else:
                    ks = [(k, 2) for k in range(0, 8, 2)] + [
                        (k, 4) for k in range(8, KD, 4)
                    ]
                for k, step in ks:
                    nc.sync.dma_start(
                        xt[:, k : k + step, :], xtp.ap()[c, :, k : k + step, :]
                    )

            # all loads trigger from the sync engine: triggers can carry
            # ring-slot waits, and on the ACT queue those block activations
            # (observed 6us PE stall at a pass boundary)
            w1_tiles = {}

            def load_w1(c, m, split=1):
                t = pw1.tile([128, KD, 128], dt.bfloat16, name=f"w1t_{c}_{m}", tag="w1")
                step = KD // split
                for j in range(0, KD, step):
                    nc.sync.dma_start(
                        t[:, j : j + step, :], w1p.ap()[m][:, j : j + step, :]
                    )
                w1_tiles[(c, m)] = t
                return t

            w2_tiles = {}

            def load_w2(c, m):
                t = pw2.tile([128, KH, 128], dt.bfloat16, name=f"w2t_{c}_{m}", tag="w2")
                nc.sync.dma_start(t[:], w2p.ap()[m])
                w2_tiles[(c, m)] = t
                return t

            # startup: the k-outer first pass needs x k-tiles plus the
            # MATCHING k-slices of all four w1 quad tiles. Interleave w1
            # slices (k-major) on the ACT engine's HWDGE port — these carry
            # no ring-slot waits, so they can't block later activations —
            # while the x pairs/quads stream from the sync engine in
            # parallel. The first slices are small so the first matmul can
            # issue as early as possible.
            emit_xt_load(0, pairs=[0])
            w1_first = {}
            for m in range(MQ):
                w1_first[m] = pw1.tile(
                    [128, KD, 128], dt.bfloat16, name=f"w1t_0_{m}", tag="w1"
                )
                w1_tiles[(0, m)] = w1_first[m]
            for j, step in ((0, 2), (2, 2), (4, 4)):
                for m in range(MQ):
                    nc.scalar.dma_start(
                        w1_first[m][:, j : j + step, :],
                        w1p.ap()[m][:, j : j + step, :],
                    )
            for j in range(8, KD, 8):
                for m in range(MQ):
                    nc.scalar.dma_start(
                        w1_first[m][:, j : j + 8, :],
                        w1p.ap()[m][:, j : j + 8, :],
                    )
            emit_xt_load(0, pairs=[2, 4, 6])
            for k in range(8, KD, 4):
                nc.sync.dma_start(
                    xts[0][:, k : k + 4, :], xtp.ap()[0, :, k : k + 4, :]
                )

            b1t = psmall.tile([128, KH], dt.float32, name="b1t")
            nc.sync.dma_start(b1t[:], b1p.ap()[:])
            b2t = psmall.tile([128, KD], dt.float32, name="b2t")
            nc.sync.dma_start(b2t[:], b2p.ap()[:])
            # resident one-hot tiles (single big DMA each); loaded after the
            # startup-critical w1/x stream, well before first use
            cpt_all = psmall.tile([128, NKG, 128], dt.bfloat16, name="cpt_all")
            ctt_all = psmall.tile([128, NKG, 128], dt.bfloat16, name="ctt_all")
            onehot_loaded = [False]

            def load_onehots():
                if not onehot_loaded[0]:
                    nc.sync.dma_start(cpt_all[:], cpp.ap()[:])
                    nc.sync.dma_start(ctt_all[:], ctpp.ap()[:])
                    onehot_loaded[0] = True

            prev_tail = [None]

            for c in range(NB):
                cs = slice(c * BC, (c + 1) * BC)
                xt = xts[c]

                # ------- phase A: hT = W1^T @ xT (+b1) ---------------------
                # Chunk 0 starts with ONE k-outer pass over m 0..3 (4 PSUM
                # banks): during the startup DMA fill each arriving x k-tile
                # feeds 4 matmuls (~0.9us of PE work), matching arrival rate.
                # Everything after runs m-serial — a k-outer pass holds all 4
                # banks to its end and then drains 4 ACTs serially, stalling
                # the next pass ~2-4us, so it is only worth it while x is
                # still streaming in.
                ht = []
                serial_start = 0
                if c == 0:
                    serial_start = MQ
                    quad = list(range(MQ))
                    w1ts = {m: w1_tiles.pop((c, m)) for m in quad}
                    load_w1(c, MQ)
                    pss = {
                        m: psum_mm.tile(
                            [128, BC], dt.float32, name=f"psA_{c}_{m}", tag="mm"
                        )
                        for m in quad
                    }
                    for k in range(KD):
                        for m in quad:
                            nc.tensor.matmul(
                                pss[m][:],
                                w1ts[m][:, k, :],
                                xt[:, k, :],
                                start=(k == 0),
                                stop=(k == KD - 1),
                            )
                    for m in quad:
                        hm = pbig.tile(
                            [128, BC], dt.bfloat16, name=f"ht_{c}_{m}", tag=f"ht{m}"
                        )
                        nc.scalar.activation(
                            hm[:], pss[m][:], AF.Identity, bias=b1t[:, m : m + 1]
                        )
                        ht.append(hm)
                for m in range(serial_start, KH):
                    w1t = w1_tiles.pop((c, m), None) or load_w1(c, m)
                    if m + 3 < KH and (c, m + 3) not in w1_tiles:
                        load_w1(c, m + 3)
                    ps = psum_mm.tile(
                        [128, BC], dt.float32, name=f"psA_{c}_{m}", tag="mm"
                    )
                    for k in range(KD):
                        nc.tensor.matmul(
                            ps[:],
                            w1t[:, k, :],
                            xt[:, k, :],
                            start=(k == 0),
                            stop=(k == KD - 1),
                        )
                    hm = pbig.tile(
                        [128, BC], dt.bfloat16, name=f"ht_{c}_{m}", tag=f"ht{m}"
                    )
                    nc.scalar.activation(
                        hm[:], ps[:], AF.Identity, bias=b1t[:, m : m + 1]
                    )
                    ht.append(hm)
                    if m == 6:
                        load_onehots()
                    if m == KH - 8:
                        load_w2(c, 0)
                    if m == KH - 4:
                        load_w2(c, 1)
                    if prev_tail[0] is not None and m in (1, 4):
                        # previous chunk's softmax tail halves: run on the PE
                        # here, long after its recip chain finished
                        prev_tail[0](half=0 if m == 1 else 1)
                        if m == 4:
                            prev_tail[0] = None

                # -------- phase B: yT = W2^T @ hT (+b2), e = exp(yT) -----
                # -------- + segment reduce / recip / broadcast / out -----
                et = [None] * KD
                spsum = {}
                r_tiles = {}
                groups_done = set()
                bcast_pending = list(range(KD))
                bcast_ready = []  # ready, emission delayed one B-group

                def emit_bcast(
                    m2, c=c, cs=cs, et=et, r_tiles=r_tiles, out_eng=None
                ):
                    gl = m_groups[m2]
                    pd = psum_d.tile(
                        [128, BC], dt.float32, name=f"pd_{c}_{m2}", tag="pd"
                    )
                    for idx, g in enumerate(gl):
                        nc.tensor.matmul(
                            pd[:],
                            ctt_all[:, kg_index[(m2, g)], :],
                            r_tiles[g][:],
                            start=(idx == 0),
                            stop=(idx == len(gl) - 1),
                        )
                    ot = pev.tile([128, BC], dt.bfloat16, name=f"ot_{c}_{m2}", tag="ot")
                    nc.vector.tensor_mul(ot[:], pd[:], et[m2][:])
                    (out_eng or nc.sync).dma_start(outp.ap()[m2][:, cs], ot[:])

                def flush_bcast(
                    max_n=None, bcast_ready=bcast_ready, emit_bcast=emit_bcast
                ):
                    n = len(bcast_ready) if max_n is None else max_n
                    for m2 in bcast_ready[:n]:
                        emit_bcast(m2)
                    del bcast_ready[:n]

                def emit_reduce(
                    k,
                    c=c,
                    et=et,
                    spsum=spsum,
                    r_tiles=r_tiles,
                    groups_done=groups_done,
                    bcast_pending=bcast_pending,
                    bcast_ready=bcast_ready,
                ):
                    for g in m_groups[k]:
                        if g not in spsum:
                            spsum[g] = psum_s.tile(
                                [128, BC], dt.float32, name=f"pss_{c}_{g}", tag="ps_s"
                            )
                        nc.tensor.matmul(
                            spsum[g][:],
                            cpt_all[:, kg_index[(k, g)], :],
                            et[k][:],
                            start=(k == k_first[g]),
                            stop=(k == k_last[g]),
                        )
                        if k == k_last[g]:
                            # reciprocal as exp(-ln(s)): two fast ACT passes
                            # (the DVE reciprocal takes 3.4us, 5x an ACT op),
                            # with the second pass narrowing to bf16 for the
                            # broadcast matmul. The empty segment row holds
                            # 2^-60 * e[feat0] (a normal float, never
                            # selected by any broadcast).
                            lg = pev.tile(
                                [128, BC], dt.float32, name=f"ls_{c}_{g}", tag="ls"
                            )
                            nc.scalar.activation(lg[:], spsum[g][:], AF.Ln)
                            rg = pbig.tile(
                                [128, BC], dt.bfloat16, name=f"r_{c}_{g}", tag=f"r{g}"
                            )
                            nc.scalar.activation(
                                rg[:], lg[:], AF.Exp, scale=-1.0
                            )
                            r_tiles[g] = rg
                            groups_done.add(g)
                            # queue feature tiles whose groups are all ready
                            still = []
                            for m2 in bcast_pending:
                                if et[m2] is not None and all(
                                    gg in groups_done for gg in m_groups[m2]
                                ):
                                    bcast_ready.append(m2)
                                else:
                                    still.append(m2)
                            bcast_pending[:] = still

                for m in range(KD):
                    if c + 1 < NB and m < KD // 2:
                        # trickle next chunk's x prefetch: one k-pair per
                        # B-group so it never bursts against the W2 stream
                        emit_xt_load(c + 1, pairs=[2 * m])
                    if m + 2 < KD and (c, m + 2) not in w2_tiles:
                        load_w2(c, m + 2)
                    if c + 1 < NB and KD - 5 <= m < KD - 1:
                        # next chunk's first w1 quad, triggered well ahead
                        load_w1(c + 1, m - (KD - 5))
                    w2t = w2_tiles.pop((c, m))
                    ps = psum_mm.tile(
                        [128, BC], dt.float32, name=f"psB_{c}_{m}", tag="mm"
                    )
                    for k in range(KH):
                        nc.tensor.matmul(
                            ps[:],
                            w2t[:, k, :],
                            ht[k][:],
                            start=(k == 0),
                            stop=(k == KH - 1),
                        )
                    em = pbig.tile(
                        [128, BC], dt.bfloat16, name=f"et_{c}_{m}", tag=f"et{m}"
                    )
                    nc.scalar.activation(em[:], ps[:], AF.Exp, bias=b2t[:, m : m + 1])
                    et[m] = em
                    # delayed work: bcasts queued >=1 B-group ago (trickled
                    # so DVE mult bursts never delay a reciprocal), then the
                    # reduce for k-tile m-1 (the lag hides ACT/DVE latency).
                    # At the last slot of the final chunk, drain everything
                    # ready — the muls overlap the remaining main matmuls.
                    flush_bcast(max_n=None if m == KD - 1 else 2)
                    if m >= 1:
                        emit_reduce(m - 1)

                # tail part 1 now: the final reduce + recip chain starts
                # immediately after the last B group
                emit_reduce(KD - 1)

                def tail(
                    half=None,
                    emit_bcast=emit_bcast,
                    bcast_pending=bcast_pending,
                    bcast_ready=bcast_ready,
                    final=(c + 1 == NB),
                ):
                    rest = bcast_ready + bcast_pending
                    if half == 0:
                        rest = rest[: (len(rest) + 1) // 2]
                    elif half == 1:
                        rest = rest[(len(rest) + 1) // 2 :]
                    for idx, m2 in enumerate(rest):
                        # in the final drain the ACT engine is idle, so
                        # alternate store triggers across both HWDGE ports;
                        # mid-kernel the ACT port is busy — keep sync only
                        emit_bcast(
                            m2,
                            out_eng=nc.scalar if (final and idx % 2) else nc.sync,
                        )
                    if half != 0:
                        del bcast_ready[:]
                        bcast_pending.clear()

                if c + 1 < NB:
                    # defer part 2: the PE executes the remaining broadcasts
                    # inside the next chunk's phase A, by which time the
                    # recip chain is long done
                    prev_tail[0] = tail
                else:
                    tail()

    _split_excess_waits(nc)
    return nc


def _pack_inputs(x, segment_ids, W1, b1, W2, b2):
    """Host-side shard + pack. Returns in_maps (one dict per core)."""
    import ml_dtypes

    bf16 = ml_dtypes.bfloat16
    seg = np.asarray(segment_ids).astype(np.int64)
    bounds, kg_pairs, k_first, _, _ = _segment_plan(seg)
    NKG = len(kg_pairs)
    barr = np.asarray(bounds)
    gof = np.searchsorted(barr, np.arange(S), side="right") - 1

    # one-hot tiles for the segment matmuls (partition-major packing)
    cp = np.zeros((NKG, 128, 128), dtype=bf16)
    ctp = np.zeros((NKG, 128, 128), dtype=bf16)
    for i, (k, g) in enumerate(kg_pairs):
        loc = seg[k * 128 : (k + 1) * 128] - barr[g]
        in_g = gof[seg[k * 128 : (k + 1) * 128]] == g
        rows = np.arange(128)
        cp[i, rows[in_g], loc[in_g]] = 1
        ctp[i, loc[in_g], rows[in_g]] = 1
    # any segment ROW with no contributing feature (empty real segments, and
    # the padding rows of groups smaller than 128) would sum to exactly 0 and
    # turn the Ln/Exp reciprocal into inf, which the broadcast matmul's 0.0
    # weights would then turn into NaN (0*inf). Plant a 2^-60 weight on
    # feature 0 of the group's first k-tile for those rows: their sums become
    # small-but-normal floats whose (finite, huge) reciprocals are multiplied
    # by exact 0.0 in every broadcast, contributing nothing.
    counts = np.bincount(seg, minlength=S)
    for g in range(len(bounds) - 1):
        i0 = kg_pairs.index((k_first[g], g))
        size_g = bounds[g + 1] - bounds[g]
        dead = [r for r in range(128) if r >= size_g or counts[barr[g] + r] == 0]
        for r in dead:
            cp[i0, 0, r] = 2.0**-60
    cpp = np.ascontiguousarray(cp.transpose(1, 0, 2))
    ctpp = np.ascontiguousarray(ctp.transpose(1, 0, 2))

    w1p = np.ascontiguousarray(
        W1.reshape(KD, 128, KH, 128).transpose(2, 1, 0, 3)
    ).astype(bf16)
    w2p = np.ascontiguousarray(
        W2.reshape(KH, 128, KD, 128).transpose(2, 1, 0, 3)
    ).astype(bf16)
    b1p = np.ascontiguousarray(b1.reshape(KH, 128).T).astype(np.float32)
    b2p = np.ascontiguousarray(b2.reshape(KD, 128).T).astype(np.float32)

    in_maps = []
    for core in range(NCORES):
        xs = x[core * BS : (core + 1) * BS]  # [BS, D]
        xtp = np.ascontiguousarray(
            xs.reshape(NB, BC, KD, 128).transpose(0, 3, 2, 1)
        ).astype(bf16)
        in_maps.append(
            {
                "xtp": xtp,
                "w1p": w1p,
                "w2p": w2p,
                "b1p": b1p,
                "b2p": b2p,
                "cpp": cpp,
                "ctpp": ctpp,
            }
        )
    return in_maps


def _unpack_outputs(results):
    """results: list (per core) of {"outp": [KD, 128, BS]} -> [B, D] f32."""
    parts = []
    for core in range(NCORES):
        outp = np.asarray(results[core]["outp"], dtype=np.float32)
        parts.append(np.ascontiguousarray(outp.transpose(2, 0, 1)).reshape(BS, D))
    return np.concatenate(parts, axis=0)


_CACHE = {}

# test harness hooks (not used in the graded path)
TRACE = False
TRACE_ALL_CORES = False
LAST_RESULT = None


def kernel(x, segment_ids, W1, b1, W2, b2):
    global LAST_RESULT
    _import_concourse()
    from concourse.bass_utils import run_bass_kernel_spmd

    key = np.asarray(segment_ids).tobytes()
    if key not in _CACHE:
        _CACHE[key] = _build_program(segment_ids)
    nc = _CACHE[key]

    in_maps = _pack_inputs(
        np.asarray(x, dtype=np.float32),
        segment_ids,
        np.asarray(W1, dtype=np.float32),
        np.asarray(b1, dtype=np.float32),
        np.asarray(W2, dtype=np.float32),
        np.asarray(b2, dtype=np.float32),
    )
    kw = {"trace_cores": list(range(NCORES))} if TRACE_ALL_CORES else {}
    res = run_bass_kernel_spmd(nc, in_maps, list(range(NCORES)), trace=TRACE, **kw)
    LAST_RESULT = res
    return _unpack_outputs(res.results)


# revision 17
# speedup vs baseline: 1.0146x; 1.0146x over previous
"""Trainium2 Bass kernel for nn_AutoEncoder_48052094108202.

  h = x @ W1 + b1          # [B, H]
  y = h @ W2 + b2          # [B, D]
  out = segmented_softmax(y, segment_ids)   # softmax over contiguous
                                            # feature segments, per row

B=8192, D=4096, H=2048, S=512 segments. Data-parallel over B across 8
NeuronCores (1024 rows/core), weights replicated.

Per-core layout: everything runs transposed (features on SBUF partitions,
batch on the free axis) so no on-device transposes are needed — the host
pre-packs x^T (and un-transposes the output). The segmented softmax is done
entirely on the tensor engine with one-hot matmuls (exact — every product is
1.0 * x):
  seg_sums   s[seg, b] = C_g^T @ e     (C one-hot features->segments)
  recip      r = exp(-ln(s))           (two fast ACT passes, bf16 out)
  broadcast  d[feat, b] = C_g @ r      (one-hot rows)
  out        = e * d                   (DVE, bf16 out)
Matmuls run in bf16 (inputs/weights rounded on host), accumulation in fp32
PSUM. exp() on the ACT engine with the bias folded in. The single empty
segment row gets a 2^-60 weight on one feature in the host-built one-hot so
its sum stays a normal float (its reciprocal row is never read).

Segment groups are 128-segment blocks, split progressively finer over the
last k-tiles so each group's drain (reduce -> recip -> broadcast -> mul ->
store) starts as soon as its k-range completes instead of queueing behind
the final matmul.

Chunk 0's phase A opens with ONE k-outer pass over m-tiles 0-3 (4 PSUM
banks): during the startup DMA fill each arriving x k-tile feeds 4 matmuls
(~864ns of PE work), matching the DMA arrival rate, so the PE does useful
work instead of warm-up padding; the matching k-slices of the four w1 tiles
stream in k-major order. Everything else runs m-serial (a k-outer pass
holds all 4 banks to its end and then drains 4 ACTs serially, stalling the
next pass). Steady-state loads trigger from the sync engine (triggers can
carry ring-slot waits; on the ACT queue those block activations and stall
the PE) — only the wait-free startup w1 slices use the ACT engine's HWDGE
port, in parallel with sync's x stream. The batch shard is processed in 2
chunks of 512 columns; output is written bf16 and upcast on host.
"""

import os
import sys

import numpy as np

# ---------------------------------------------------------------- constants
B, D, H, S = 8192, 4096, 2048, 512
NCORES = 8
BS = B // NCORES  # 1024 batch rows per core
NB = 2  # chunks per core
BC = BS // NB  # 512 batch rows per chunk
KD = D // 128  # 32 k-tiles over D
KH = H // 128  # 16 k-tiles over H
MQ = 4  # phase-A m-tiles per k-outer pass

_WAIT_LIMIT = 1  # walrus CoreV3 accepts 1 sync-wait per instruction


def _import_concourse():
    try:
        import concourse  # noqa: F401
    except ImportError:
        for p in ("/opt/trn_rl_repo", "/root/.axon_site/_ro/trn_rl_repo"):
            if os.path.isdir(p) and p not in sys.path:
                sys.path.insert(0, p)
        import concourse  # noqa: F401


def _split_excess_waits(nc, limit=_WAIT_LIMIT):
    """walrus rejects instructions carrying more than one sync-wait; hoist
    extras onto preceding NOPs on the same engine (same semantics: blocking
    waits on one sequencer, order irrelevant)."""
    import bass_rust

    engines = nc.engines
    for fn in nc.m.functions:
        for bb in fn.blocks:
            insts = bb.instructions
            i = 0
            while i < len(insts):
                inst = insts[i]
                si = inst.sync_info
                waits = list(si.on_wait) if si and si.on_wait else []
                if len(waits) > limit:
                    overflow, keep = waits[:-limit], waits[-limit:]
                    si.on_wait = keep
                    pos = i
                    for j in range(0, len(overflow), limit):
                        nop = engines[inst.engine].nop(
                            nofuse=True, hint="wait_split"
                        ).ins
                        for b2 in fn.blocks:
                            lst = b2.instructions
                            if nop in lst:
                                lst.remove(nop)
                        nop.sync_info = bass_rust.SyncInfo(
                            on_wait=overflow[j : j + limit], on_update=[]
                        )
                        insts.insert(pos, nop)
                        pos += 1
                        i += 1
                i += 1


def _segment_plan(seg):
    """Static plan from the (sorted) segment ids.

    Groups are 128-aligned segment blocks, with the final block split at the
    first segment of the last k-tile so the kernel's drain group is tiny.

    Returns (bounds, kg_pairs, k_first, k_last, m_groups) where bounds[g] is
    the first segment id of group g (bounds has n_groups+1 entries).
    """
    seg = np.asarray(seg).astype(np.int64)
    assert seg.shape == (D,)
    bounds = [0, 128, 256, 384]
    # fine-grained groups over the last k-tiles: each completes (and its
    # broadcasts start) progressively instead of queueing one big group's
    # drain behind the final matmul
    for k in (26, 28, 30, 31):
        s_k = int(seg[k * 128])  # first segment of k-tile k
        if bounds[-1] < s_k < S:
            bounds.append(s_k)
    bounds.append(S)
    bounds = sorted(set(bounds))
    gof = np.searchsorted(np.asarray(bounds), np.arange(S), side="right") - 1

    kg_pairs = []
    for k in range(KD):
        gs = np.unique(gof[seg[k * 128 : (k + 1) * 128]])
        for g in gs:
            kg_pairs.append((k, int(g)))
    k_first = {}
    k_last = {}
    for k, g in kg_pairs:
        k_first.setdefault(g, k)
        k_last[g] = k
    m_groups = {}
    for k, g in kg_pairs:
        m_groups.setdefault(k, []).append(g)
    return bounds, kg_pairs, k_first, k_last, m_groups


def _build_program(seg):
    """Build the (SPMD, per-core) Bass program. Same program on all cores."""
    _import_concourse()
    import concourse.bass as bass
    import concourse.mybir as mybir
    from concourse import tile

    dt = mybir.dt
    AF = mybir.ActivationFunctionType

    bounds, kg_pairs, k_first, k_last, m_groups = _segment_plan(seg)
    NKG = len(kg_pairs)
    kg_index = {pair: i for i, pair in enumerate(kg_pairs)}

    nc = bass.Bass("TRN2", target_bir_lowering=False, debug=False)

    xtp = nc.dram_tensor("xtp", [NB, 128, KD, BC], dt.bfloat16, kind="ExternalInput")
    w1p = nc.dram_tensor("w1p", [KH, 128, KD, 128], dt.bfloat16, kind="ExternalInput")
    w2p = nc.dram_tensor("w2p", [KD, 128, KH, 128], dt.bfloat16, kind="ExternalInput")
    b1p = nc.dram_tensor("b1p", [128, KH], dt.float32, kind="ExternalInput")
    b2p = nc.dram_tensor("b2p", [128, KD], dt.float32, kind="ExternalInput")
    # one-hot tiles, partition-major so each loads as a single DMA
    cpp = nc.dram_tensor("cpp", [128, NKG, 128], dt.bfloat16, kind="ExternalInput")
    ctpp = nc.dram_tensor("ctpp", [128, NKG, 128], dt.bfloat16, kind="ExternalInput")
    outp = nc.dram_tensor("outp", [KD, 128, BS], dt.bfloat16, kind="ExternalOutput")

    with tile.TileContext(nc) as tc:
        with (
            tc.tile_pool(name="pbig", bufs=1) as pbig,
            tc.tile_pool(name="pw1", bufs=5) as pw1,
            tc.tile_pool(name="pw2", bufs=3) as pw2,
            tc.tile_pool(name="pev", bufs=3) as pev,
            tc.tile_pool(name="psmall", bufs=1) as psmall,
            tc.tile_pool(name="psum_mm", bufs=MQ, space="PSUM") as psum_mm,
            tc.tile_pool(name="psum_s", bufs=2, space="PSUM") as psum_s,
            tc.tile_pool(name="psum_d", bufs=2, space="PSUM") as psum_d,
        ):
            xts = {}

            def emit_xt_load(c, pairs=None):
                if c not in xts:
                    xts[c] = pbig.tile(
                        [128, KD, BC], dt.bfloat16, name=f"xt{c}", tag="xt", bufs=2
                    )
                xt = xts[c]
                if pairs is not None:
                    ks = [(k, 2) for k in pairs]
                else:
                    ks = [(k, 2) for k in range(0, 8, 2)] + [
                        (k, 4) for k in range(8, KD, 4)
                    ]
                for k, step in ks:
                    nc.sync.dma_start(
                        xt[:, k : k + step, :], xtp.ap()[c, :, k : k + step, :]
                    )

            # all loads trigger from the sync engine: triggers can carry
            # ring-slot waits, and on the ACT queue those block activations
            # (observed 6us PE stall at a pass boundary)
            w1_tiles = {}

            def load_w1(c, m, split=1):
                t = pw1.tile([128, KD, 128], dt.bfloat16, name=f"w1t_{c}_{m}", tag="w1")
                step = KD // split
                for j in range(0, KD, step):
                    nc.sync.dma_start(
                        t[:, j : j + step, :], w1p.ap()[m][:, j : j + step, :]
                    )
                w1_tiles[(c, m)] = t
                return t

            w2_tiles = {}

            def load_w2(c, m):
                t = pw2.tile([128, KH, 128], dt.bfloat16, name=f"w2t_{c}_{m}", tag="w2")
                nc.sync.dma_start(t[:], w2p.ap()[m])
                w2_tiles[(c, m)] = t
                return t

            # startup: the k-outer first pass needs x k-tiles plus the
            # MATCHING k-slices of all four w1 quad tiles. Interleave w1
            # slices (k-major) on the ACT engine's HWDGE port — these carry
            # no ring-slot waits, so they can't block later activations —
            # while the x pairs/quads stream from the sync engine in
            # parallel. The first slices are small so the first matmul can
            # issue as early as possible.
            emit_xt_load(0, pairs=[0])
            w1_first = {}
            for m in range(MQ):
                w1_first[m] = pw1.tile(
                    [128, KD, 128], dt.bfloat16, name=f"w1t_0_{m}", tag="w1"
                )
                w1_tiles[(0, m)] = w1_first[m]
            for j, step in ((0, 4), (4, 4)):
                for m in range(MQ):
                    nc.scalar.dma_start(
                        w1_first[m][:, j : j + step, :],
                        w1p.ap()[m][:, j : j + step, :],
                    )
            for j in range(8, KD, 8):
                for m in range(MQ):
                    nc.scalar.dma_start(
                        w1_first[m][:, j : j + 8, :],
                        w1p.ap()[m][:, j : j + 8, :],
                    )
            emit_xt_load(0, pairs=[2, 4, 6])
            for k in range(8, KD, 4):
                nc.sync.dma_start(
                    xts[0][:, k : k + 4, :], xtp.ap()[0, :, k : k + 4, :]
                )

            b1t = psmall.tile([128, KH], dt.float32, name="b1t")
            nc.sync.dma_start(b1t[:], b1p.ap()[:])
            b2t = psmall.tile([128, KD], dt.float32, name="b2t")
            nc.sync.dma_start(b2t[:], b2p.ap()[:])
            # resident one-hot tiles (single big DMA each); loaded after the
            # startup-critical w1/x stream, well before first use
            cpt_all = psmall.tile([128, NKG, 128], dt.bfloat16, name="cpt_all")
            ctt_all = psmall.tile([128, NKG, 128], dt.bfloat16, name="ctt_all")
            onehot_loaded = [False]

            def load_onehots():
                if not onehot_loaded[0]:
                    nc.sync.dma_start(cpt_all[:], cpp.ap()[:])
                    nc.sync.dma_start(ctt_all[:], ctpp.ap()[:])
                    onehot_loaded[0] = True

            prev_tail = [None]

            for c in range(NB):
                cs = slice(c * BC, (c + 1) * BC)
                xt = xts[c]

                # ------- phase A: hT = W1^T @ xT (+b1) ---------------------
                # Chunk 0 starts with ONE k-outer pass over m 0..3 (4 PSUM
                # banks): during the startup DMA fill each arriving x k-tile
                # feeds 4 matmuls (~0.9us of PE work), matching arrival rate.
                # Everything after runs m-serial — a k-outer pass holds all 4
                # banks to its end and then drains 4 ACTs serially, stalling
                # the next pass ~2-4us, so it is only worth it while x is
                # still streaming in.
                ht = []
                serial_start = 0
                if c == 0:
                    serial_start = MQ
                    quad = list(range(MQ))
                    w1ts = {m: w1_tiles.pop((c, m)) for m in quad}
                    load_w1(c, MQ)
                    pss = {
                        m: psum_mm.tile(
                            [128, BC], dt.float32, name=f"psA_{c}_{m}", tag="mm"
                        )
                        for m in quad
                    }
                    for k in range(KD):
                        for m in quad:
                            nc.tensor.matmul(
                                pss[m][:],
                                w1ts[m][:, k, :],
                                xt[:, k, :],
                                start=(k == 0),
                                stop=(k == KD - 1),
                            )
                    for m in quad:
                        hm = pbig.tile(
                            [128, BC], dt.bfloat16, name=f"ht_{c}_{m}", tag=f"ht{m}"
                        )
                        nc.scalar.activation(
                            hm[:], pss[m][:], AF.Identity, bias=b1t[:, m : m + 1]
                        )
                        ht.append(hm)
                for m in range(serial_start, KH):
                    w1t = w1_tiles.pop((c, m), None) or load_w1(c, m)
                    if m + 3 < KH and (c, m + 3) not in w1_tiles:
                        load_w1(c, m + 3)
                    ps = psum_mm.tile(
                        [128, BC], dt.float32, name=f"psA_{c}_{m}", tag="mm"
                    )
                    for k in range(KD):
                        nc.tensor.matmul(
                            ps[:],
                            w1t[:, k, :],
                            xt[:, k, :],
                            start=(k == 0),
                            stop=(k == KD - 1),
                        )
                    hm = pbig.tile(
                        [128, BC], dt.bfloat16, name=f"ht_{c}_{m}", tag=f"ht{m}"
                    )
                    nc.scalar.activation(
                        hm[:], ps[:], AF.Identity, bias=b1t[:, m : m + 1]
                    )
                    ht.append(hm)
                    if m == 6:
                        load_onehots()
                    if m == KH - 8:
                        load_w2(c, 0)
                    if m == KH - 4:
                        load_w2(c, 1)
                    if prev_tail[0] is not None and m in (1, 4):
                        # previous chunk's softmax tail halves: run on the PE
                        # here, long after its recip chain finished
                        prev_tail[0](half=0 if m == 1 else 1)
                        if m == 4:
                            prev_tail[0] = None

                # -------- phase B: yT = W2^T @ hT (+b2), e = exp(yT) -----
                # -------- + segment reduce / recip / broadcast / out -----
                et = [None] * KD
                spsum = {}
                r_tiles = {}
                groups_done = set()
                bcast_pending = list(range(KD))
                bcast_ready = []  # ready, emission delayed one B-group

                def emit_bcast(
                    m2, c=c, cs=cs, et=et, r_tiles=r_tiles, out_eng=None
                ):
                    gl = m_groups[m2]
                    pd = psum_d.tile(
                        [128, BC], dt.float32, name=f"pd_{c}_{m2}", tag="pd"
                    )
                    for idx, g in enumerate(gl):
                        nc.tensor.matmul(
                            pd[:],
                            ctt_all[:, kg_index[(m2, g)], :],
                            r_tiles[g][:],
                            start=(idx == 0),
                            stop=(idx == len(gl) - 1),
                        )
                    ot = pev.tile([128, BC], dt.bfloat16, name=f"ot_{c}_{m2}", tag="ot")
                    nc.vector.tensor_mul(ot[:], pd[:], et[m2][:])
                    (out_eng or nc.sync).dma_start(outp.ap()[m2][:, cs], ot[:])

                def flush_bcast(
                    max_n=None, bcast_ready=bcast_ready, emit_bcast=emit_bcast
                ):
                    n = len(bcast_ready) if max_n is None else max_n
                    for m2 in bcast_ready[:n]:
                        emit_bcast(m2)
                    del bcast_ready[:n]

                def emit_reduce(
                    k,
                    c=c,
                    et=et,
                    spsum=spsum,
                    r_tiles=r_tiles,
                    groups_done=groups_done,
                    bcast_pending=bcast_pending,
                    bcast_ready=bcast_ready,
                ):
                    for g in m_groups[k]:
                        if g not in spsum:
                            spsum[g] = psum_s.tile(
                                [128, BC], dt.float32, name=f"pss_{c}_{g}", tag="ps_s"
                            )
                        nc.tensor.matmul(
                            spsum[g][:],
                            cpt_all[:, kg_index[(k, g)], :],
                            et[k][:],
                            start=(k == k_first[g]),
                            stop=(k == k_last[g]),
                        )
                        if k == k_last[g]:
                            # reciprocal as exp(-ln(s)): two fast ACT passes
                            # (the DVE reciprocal takes 3.4us, 5x an ACT op),
                            # with the second pass narrowing to bf16 for the
                            # broadcast matmul. The empty segment row holds
                            # 2^-60 * e[feat0] (a normal float, never
                            # selected by any broadcast).
                            lg = pev.tile(
                                [128, BC], dt.float32, name=f"ls_{c}_{g}", tag="ls"
                            )
                            nc.scalar.activation(lg[:], spsum[g][:], AF.Ln)
                            rg = pbig.tile(
                                [128, BC], dt.bfloat16, name=f"r_{c}_{g}", tag=f"r{g}"
                            )
                            nc.scalar.activation(
                                rg[:], lg[:], AF.Exp, scale=-1.0
                            )
                            r_tiles[g] = rg
                            groups_done.add(g)
                            # queue feature tiles whose groups are all ready
                            still = []
                            for m2 in bcast_pending:
                                if et[m2] is not None and all(
                                    gg in groups_done for gg in m_groups[m2]
                                ):
                                    bcast_ready.append(m2)
                                else:
                                    still.append(m2)
                            bcast_pending[:] = still

                for m in range(KD):
                    if c + 1 < NB and m < KD // 2:
                        # trickle next chunk's x prefetch: one k-pair per
                        # B-group so it never bursts against the W2 stream
                        emit_xt_load(c + 1, pairs=[2 * m])
                    if m + 2 < KD and (c, m + 2) not in w2_tiles:
                        load_w2(c, m + 2)
                    if c + 1 < NB and KD - 5 <= m < KD - 1:
                        # next chunk's first w1 quad, triggered well ahead
                        load_w1(c + 1, m - (KD - 5))
                    w2t = w2_tiles.pop((c, m))
                    ps = psum_mm.tile(
                        [128, BC], dt.float32, name=f"psB_{c}_{m}", tag="mm"
                    )
                    for k in range(KH):
                        nc.tensor.matmul(
                            ps[:],
                            w2t[:, k, :],
                            ht[k][:],
                            start=(k == 0),
                            stop=(k == KH - 1),
                        )
                    em = pbig.tile(
                        [128, BC], dt.bfloat16, name=f"et_{c}_{m}", tag=f"et{m}"
                    )
                    nc.scalar.activation(em[:], ps[:], AF.Exp, bias=b2t[:, m : m + 1])
                    et[m] = em
                    # delayed work: bcasts queued >=1 B-group ago (trickled
                    # so DVE mult bursts never delay a reciprocal), then the
                    # reduce for k-tile m-1 (the lag hides ACT/DVE latency).
                    # At the last slot of the final chunk, drain everything
                    # ready — the muls overlap the remaining main matmuls.
                    flush_bcast(max_n=None if m == KD - 1 else 2)
                    if m >= 1:
                        emit_reduce(m - 1)

                # tail part 1 now: the final reduce + recip chain starts
                # immediately after the last B group
                emit_reduce(KD - 1)

                def tail(
                    half=None,
                    emit_bcast=emit_bcast,
                    bcast_pending=bcast_pending,
                    bcast_ready=bcast_ready,
                    final=(c + 1 == NB),
                ):
                    rest = bcast_ready + bcast_pending
                    if half == 0:
                        rest = rest[: (len(rest) + 1) // 2]
                    elif half == 1:
                        rest = rest[(len(rest) + 1) // 2 :]
                    for idx, m2 in enumerate(rest):
                        # in the final drain the ACT engine is idle, so
                        # alternate store triggers across both HWDGE ports;
                        # mid-kernel the ACT port is busy — keep sync only
                        emit_bcast(
                            m2,
                            out_eng=nc.scalar if (final and idx % 2) else nc.sync,
                        )
                    if half != 0:
                        del bcast_ready[:]
                        bcast_pending.clear()

                if c + 1 < NB:
                    # defer part 2: the PE executes the remaining broadcasts
                    # inside the next chunk's phase A, by which time the
                    # recip chain is long done
                    prev_tail[0] = tail
                else:
                    tail()

    _split_excess_waits(nc)
    return nc


def _pack_inputs(x, segment_ids, W1, b1, W2, b2):
    """Host-side shard + pack. Returns in_maps (one dict per core)."""
    import ml_dtypes

    bf16 = ml_dtypes.bfloat16
    seg = np.asarray(segment_ids).astype(np.int64)
    bounds, kg_pairs, k_first, _, _ = _segment_plan(seg)
    NKG = len(kg_pairs)
    barr = np.asarray(bounds)
    gof = np.searchsorted(barr, np.arange(S), side="right") - 1

    # one-hot tiles for the segment matmuls (partition-major packing)
    cp = np.zeros((NKG, 128, 128), dtype=bf16)
    ctp = np.zeros((NKG, 128, 128), dtype=bf16)
    for i, (k, g) in enumerate(kg_pairs):
        loc = seg[k * 128 : (k + 1) * 128] - barr[g]
        in_g = gof[seg[k * 128 : (k + 1) * 128]] == g
        rows = np.arange(128)
        cp[i, rows[in_g], loc[in_g]] = 1
        ctp[i, loc[in_g], rows[in_g]] = 1
    # any segment ROW with no contributing feature (empty real segments, and
    # the padding rows of groups smaller than 128) would sum to exactly 0 and
    # turn the Ln/Exp reciprocal into inf, which the broadcast matmul's 0.0
    # weights would then turn into NaN (0*inf). Plant a 2^-60 weight on
    # feature 0 of the group's first k-tile for those rows: their sums become
    # small-but-normal floats whose (finite, huge) reciprocals are multiplied
    # by exact 0.0 in every broadcast, contributing nothing.
    counts = np.bincount(seg, minlength=S)
    for g in range(len(bounds) - 1):
        i0 = kg_pairs.index((k_first[g], g))
        size_g = bounds[g + 1] - bounds[g]
        dead = [r for r in range(128) if r >= size_g or counts[barr[g] + r] == 0]
        for r in dead:
            cp[i0, 0, r] = 2.0**-60
    cpp = np.ascontiguousarray(cp.transpose(1, 0, 2))
    ctpp = np.ascontiguousarray(ctp.transpose(1, 0, 2))

    w1p = np.ascontiguousarray(
        W1.reshape(KD, 128, KH, 128).transpose(2, 1, 0, 3)
    ).astype(bf16)
    w2p = np.ascontiguousarray(
        W2.reshape(KH, 128, KD, 128).transpose(2, 1, 0, 3)
    ).astype(bf16)
    b1p = np.ascontiguousarray(b1.reshape(KH, 128).T).astype(np.float32)
    b2p = np.ascontiguousarray(b2.reshape(KD, 128).T).astype(np.float32)

    in_maps = []
    for core in range(NCORES):
        xs = x[core * BS : (core + 1) * BS]  # [BS, D]
        xtp = np.ascontiguousarray(
            xs.reshape(NB, BC, KD, 128).transpose(0, 3, 2, 1)
        ).astype(bf16)
        in_maps.append(
            {
                "xtp": xtp,
                "w1p": w1p,
                "w2p": w2p,
                "b1p": b1p,
                "b2p": b2p,
                "cpp": cpp,
                "ctpp": ctpp,
            }
        )
    return in_maps


def _unpack_outputs(results):
    """results: list (per core) of {"outp": [KD, 128, BS]} -> [B, D] f32."""
    parts = []
    for core in range(NCORES):
        outp = np.asarray(results[core]["outp"], dtype=np.float32)
        parts.append(np.ascontiguousarray(outp.transpose(2, 0, 1)).reshape(BS, D))
    return np.concatenate(parts, axis=0)


_CACHE = {}

# test harness hooks (not used in the graded path)
TRACE = False
TRACE_ALL_CORES = False
LAST_RESULT = None


def kernel(x, segment_ids, W1, b1, W2, b2):
    global LAST_RESULT
    _import_concourse()
    from concourse.bass_utils import run_bass_kernel_spmd

    key = np.asarray(segment_ids).tobytes()
    if key not in _CACHE:
        _CACHE[key] = _build_program(segment_ids)
    nc = _CACHE[key]

    in_maps = _pack_inputs(
        np.asarray(x, dtype=np.float32),
        segment_ids,
        np.asarray(W1, dtype=np.float32),
        np.asarray(b1, dtype=np.float32),
        np.asarray(W2, dtype=np.float32),
        np.asarray(b2, dtype=np.float32),
    )
    kw = {"trace_cores": list(range(NCORES))} if TRACE_ALL_CORES else {}
    res = run_bass_kernel_spmd(nc, in_maps, list(range(NCORES)), trace=TRACE, **kw)
    LAST_RESULT = res
    return _unpack_outputs(res.results)


# revision 18
# speedup vs baseline: 1.0177x; 1.0030x over previous
"""Trainium2 Bass kernel for nn_AutoEncoder_48052094108202.

  h = x @ W1 + b1          # [B, H]
  y = h @ W2 + b2          # [B, D]
  out = segmented_softmax(y, segment_ids)   # softmax over contiguous
                                            # feature segments, per row

B=8192, D=4096, H=2048, S=512 segments. Data-parallel over B across 8
NeuronCores (1024 rows/core), weights replicated.

Per-core layout: everything runs transposed (features on SBUF partitions,
batch on the free axis) so no on-device transposes are needed — the host
pre-packs x^T (and un-transposes the output). The segmented softmax is done
entirely on the tensor engine with one-hot matmuls (exact — every product is
1.0 * x):
  seg_sums   s[seg, b] = C_g^T @ e     (C one-hot features->segments)
  recip      r = exp(-ln(s))           (two fast ACT passes, bf16 out)
  broadcast  d[feat, b] = C_g @ r      (one-hot rows)
  out        = e * d                   (DVE, bf16 out)
Matmuls run in bf16 (inputs/weights rounded on host), accumulation in fp32
PSUM. exp() on the ACT engine with the bias folded in. The single empty
segment row gets a 2^-60 weight on one feature in the host-built one-hot so
its sum stays a normal float (its reciprocal row is never read).

Segment groups are 128-segment blocks, split progressively finer over the
last k-tiles so each group's drain (reduce -> recip -> broadcast -> mul ->
store) starts as soon as its k-range completes instead of queueing behind
the final matmul.

Chunk 0's phase A opens with ONE k-outer pass over m-tiles 0-3 (4 PSUM
banks): during the startup DMA fill each arriving x k-tile feeds 4 matmuls
(~864ns of PE work), matching the DMA arrival rate, so the PE does useful
work instead of warm-up padding; the matching k-slices of the four w1 tiles
stream in k-major order. Everything else runs m-serial (a k-outer pass
holds all 4 banks to its end and then drains 4 ACTs serially, stalling the
next pass). Steady-state loads trigger from the sync engine (triggers can
carry ring-slot waits; on the ACT queue those block activations and stall
the PE) — only the wait-free startup w1 slices use the ACT engine's HWDGE
port, in parallel with sync's x stream. The batch shard is processed in 2
chunks of 512 columns; output is written bf16 and upcast on host.
"""

import os
import sys

import numpy as np

# ---------------------------------------------------------------- constants
B, D, H, S = 8192, 4096, 2048, 512
NCORES = 8
BS = B // NCORES  # 1024 batch rows per core
NB = 2  # chunks per core
BC = BS // NB  # 512 batch rows per chunk
KD = D // 128  # 32 k-tiles over D
KH = H // 128  # 16 k-tiles over H
MQ = 4  # phase-A m-tiles per k-outer pass

_WAIT_LIMIT = 1  # walrus CoreV3 accepts 1 sync-wait per instruction


def _import_concourse():
    try:
        import concourse  # noqa: F401
    except ImportError:
        for p in ("/opt/trn_rl_repo", "/root/.axon_site/_ro/trn_rl_repo"):
            if os.path.isdir(p) and p not in sys.path:
                sys.path.insert(0, p)
        import concourse  # noqa: F401


def _split_excess_waits(nc, limit=_WAIT_LIMIT):
    """walrus rejects instructions carrying more than one sync-wait; hoist
    extras onto preceding NOPs on the same engine (same semantics: blocking
    waits on one sequencer, order irrelevant)."""
    import bass_rust

    engines = nc.engines
    for fn in nc.m.functions:
        for bb in fn.blocks:
            insts = bb.instructions
            i = 0
            while i < len(insts):
                inst = insts[i]
                si = inst.sync_info
                waits = list(si.on_wait) if si and si.on_wait else []
                if len(waits) > limit:
                    overflow, keep = waits[:-limit], waits[-limit:]
                    si.on_wait = keep
                    pos = i
                    for j in range(0, len(overflow), limit):
                        nop = engines[inst.engine].nop(
                            nofuse=True, hint="wait_split"
                        ).ins
                        for b2 in fn.blocks:
                            lst = b2.instructions
                            if nop in lst:
                                lst.remove(nop)
                        nop.sync_info = bass_rust.SyncInfo(
                            on_wait=overflow[j : j + limit], on_update=[]
                        )
                        insts.insert(pos, nop)
                        pos += 1
                        i += 1
                i += 1


def _segment_plan(seg):
    """Static plan from the (sorted) segment ids.

    Groups are 128-aligned segment blocks, with the final block split at the
    first segment of the last k-tile so the kernel's drain group is tiny.

    Returns (bounds, kg_pairs, k_first, k_last, m_groups) where bounds[g] is
    the first segment id of group g (bounds has n_groups+1 entries).
    """
    seg = np.asarray(seg).astype(np.int64)
    assert seg.shape == (D,)
    bounds = [0, 128, 256, 384]
    # fine-grained groups over the last k-tiles: each completes (and its
    # broadcasts start) progressively instead of queueing one big group's
    # drain behind the final matmul
    for k in (26, 28, 30, 31):
        s_k = int(seg[k * 128])  # first segment of k-tile k
        if bounds[-1] < s_k < S:
            bounds.append(s_k)
    bounds.append(S)
    bounds = sorted(set(bounds))
    gof = np.searchsorted(np.asarray(bounds), np.arange(S), side="right") - 1

    kg_pairs = []
    for k in range(KD):
        gs = np.unique(gof[seg[k * 128 : (k + 1) * 128]])
        for g in gs:
            kg_pairs.append((k, int(g)))
    k_first = {}
    k_last = {}
    for k, g in kg_pairs:
        k_first.setdefault(g, k)
        k_last[g] = k
    m_groups = {}
    for k, g in kg_pairs:
        m_groups.setdefault(k, []).append(g)
    return bounds, kg_pairs, k_first, k_last, m_groups


def _build_program(seg):
    """Build the (SPMD, per-core) Bass program. Same program on all cores."""
    _import_concourse()
    import concourse.bass as bass
    import concourse.mybir as mybir
    from concourse import tile

    dt = mybir.dt
    AF = mybir.ActivationFunctionType

    bounds, kg_pairs, k_first, k_last, m_groups = _segment_plan(seg)
    NKG = len(kg_pairs)
    kg_index = {pair: i for i, pair in enumerate(kg_pairs)}

    nc = bass.Bass("TRN2", target_bir_lowering=False, debug=False)

    xtp = nc.dram_tensor("xtp", [NB, 128, KD, BC], dt.bfloat16, kind="ExternalInput")
    w1p = nc.dram_tensor("w1p", [KH, 128, KD, 128], dt.bfloat16, kind="ExternalInput")
    w2p = nc.dram_tensor("w2p", [KD, 128, KH, 128], dt.bfloat16, kind="ExternalInput")
    b1p = nc.dram_tensor("b1p", [128, KH], dt.float32, kind="ExternalInput")
    b2p = nc.dram_tensor("b2p", [128, KD], dt.float32, kind="ExternalInput")
    # one-hot tiles, partition-major so each loads as a single DMA
    cpp = nc.dram_tensor("cpp", [128, NKG, 128], dt.bfloat16, kind="ExternalInput")
    ctpp = nc.dram_tensor("ctpp", [128, NKG, 128], dt.bfloat16, kind="ExternalInput")
    outp = nc.dram_tensor("outp", [KD, 128, BS], dt.bfloat16, kind="ExternalOutput")

    with tile.TileContext(nc) as tc:
        with (
            tc.tile_pool(name="pbig", bufs=1) as pbig,
            tc.tile_pool(name="pw1", bufs=5) as pw1,
            tc.tile_pool(name="pw2", bufs=3) as pw2,
            tc.tile_pool(name="pev", bufs=3) as pev,
            tc.tile_pool(name="psmall", bufs=1) as psmall,
            tc.tile_pool(name="psum_mm", bufs=MQ, space="PSUM") as psum_mm,
            tc.tile_pool(name="psum_s", bufs=2, space="PSUM") as psum_s,
            tc.tile_pool(name="psum_d", bufs=2, space="PSUM") as psum_d,
        ):
            xts = {}

            def emit_xt_load(c, pairs=None):
                if c not in xts:
                    xts[c] = pbig.tile(
                        [128, KD, BC], dt.bfloat16, name=f"xt{c}", tag="xt", bufs=2
                    )
                xt = xts[c]
                if pairs is not None:
                    ks = [(k, 2) for k in pairs]
                else:
                    ks = [(k, 2) for k in range(0, 8, 2)] + [
                        (k, 4) for k in range(8, KD, 4)
                    ]
                for k, step in ks:
                    nc.sync.dma_start(
                        xt[:, k : k + step, :], xtp.ap()[c, :, k : k + step, :]
                    )

            # all loads trigger from the sync engine: triggers can carry
            # ring-slot waits, and on the ACT queue those block activations
            # (observed 6us PE stall at a pass boundary)
            w1_tiles = {}

            def load_w1(c, m, split=1):
                t = pw1.tile([128, KD, 128], dt.bfloat16, name=f"w1t_{c}_{m}", tag="w1")
                step = KD // split
                for j in range(0, KD, step):
                    nc.sync.dma_start(
                        t[:, j : j + step, :], w1p.ap()[m][:, j : j + step, :]
                    )
                w1_tiles[(c, m)] = t
                return t

            w2_tiles = {}

            def load_w2(c, m):
                t = pw2.tile([128, KH, 128], dt.bfloat16, name=f"w2t_{c}_{m}", tag="w2")
                nc.sync.dma_start(t[:], w2p.ap()[m])
                w2_tiles[(c, m)] = t
                return t

            # startup: the k-outer first pass needs x k-tiles plus the
            # MATCHING k-slices of all four w1 quad tiles. Interleave w1
            # slices (k-major) on the ACT engine's HWDGE port — these carry
            # no ring-slot waits, so they can't block later activations —
            # while the x pairs/quads stream from the sync engine in
            # parallel. The first slices are small so the first matmul can
            # issue as early as possible.
            # first two x k-tiles load singly: the very first matmul waits
            # only on a 128KB transfer instead of a 256KB pair
            emit_xt_load(0, pairs=[])
            nc.sync.dma_start(xts[0][:, 0:1, :], xtp.ap()[0, :, 0:1, :])
            nc.sync.dma_start(xts[0][:, 1:2, :], xtp.ap()[0, :, 1:2, :])
            w1_first = {}
            for m in range(MQ):
                w1_first[m] = pw1.tile(
                    [128, KD, 128], dt.bfloat16, name=f"w1t_0_{m}", tag="w1"
                )
                w1_tiles[(0, m)] = w1_first[m]
            for j, step in ((0, 4), (4, 4)):
                for m in range(MQ):
                    nc.scalar.dma_start(
                        w1_first[m][:, j : j + step, :],
                        w1p.ap()[m][:, j : j + step, :],
                    )
            for j in range(8, KD, 8):
                for m in range(MQ):
                    nc.scalar.dma_start(
                        w1_first[m][:, j : j + 8, :],
                        w1p.ap()[m][:, j : j + 8, :],
                    )
            emit_xt_load(0, pairs=[2, 4, 6])
            for k in range(8, KD, 4):
                nc.sync.dma_start(
                    xts[0][:, k : k + 4, :], xtp.ap()[0, :, k : k + 4, :]
                )

            b1t = psmall.tile([128, KH], dt.float32, name="b1t")
            nc.sync.dma_start(b1t[:], b1p.ap()[:])
            b2t = psmall.tile([128, KD], dt.float32, name="b2t")
            nc.sync.dma_start(b2t[:], b2p.ap()[:])
            # resident one-hot tiles (single big DMA each); loaded after the
            # startup-critical w1/x stream, well before first use
            cpt_all = psmall.tile([128, NKG, 128], dt.bfloat16, name="cpt_all")
            ctt_all = psmall.tile([128, NKG, 128], dt.bfloat16, name="ctt_all")
            onehot_loaded = [False]

            def load_onehots():
                if not onehot_loaded[0]:
                    nc.sync.dma_start(cpt_all[:], cpp.ap()[:])
                    nc.sync.dma_start(ctt_all[:], ctpp.ap()[:])
                    onehot_loaded[0] = True

            prev_tail = [None]

            for c in range(NB):
                cs = slice(c * BC, (c + 1) * BC)
                xt = xts[c]

                # ------- phase A: hT = W1^T @ xT (+b1) ---------------------
                # Chunk 0 starts with ONE k-outer pass over m 0..3 (4 PSUM
                # banks): during the startup DMA fill each arriving x k-tile
                # feeds 4 matmuls (~0.9us of PE work), matching arrival rate.
                # Everything after runs m-serial — a k-outer pass holds all 4
                # banks to its end and then drains 4 ACTs serially, stalling
                # the next pass ~2-4us, so it is only worth it while x is
                # still streaming in.
                ht = []
                serial_start = 0
                if c == 0:
                    serial_start = MQ
                    quad = list(range(MQ))
                    w1ts = {m: w1_tiles.pop((c, m)) for m in quad}
                    load_w1(c, MQ)
                    pss = {
                        m: psum_mm.tile(
                            [128, BC], dt.float32, name=f"psA_{c}_{m}", tag="mm"
                        )
                        for m in quad
                    }
                    for k in range(KD):
                        for m in quad:
                            nc.tensor.matmul(
                                pss[m][:],
                                w1ts[m][:, k, :],
                                xt[:, k, :],
                                start=(k == 0),
                                stop=(k == KD - 1),
                            )
                    for m in quad:
                        hm = pbig.tile(
                            [128, BC], dt.bfloat16, name=f"ht_{c}_{m}", tag=f"ht{m}"
                        )
                        nc.scalar.activation(
                            hm[:], pss[m][:], AF.Identity, bias=b1t[:, m : m + 1]
                        )
                        ht.append(hm)
                for m in range(serial_start, KH):
                    w1t = w1_tiles.pop((c, m), None) or load_w1(c, m)
                    if m + 3 < KH and (c, m + 3) not in w1_tiles:
                        load_w1(c, m + 3)
                    ps = psum_mm.tile(
                        [128, BC], dt.float32, name=f"psA_{c}_{m}", tag="mm"
                    )
                    for k in range(KD):
                        nc.tensor.matmul(
                            ps[:],
                            w1t[:, k, :],
                            xt[:, k, :],
                            start=(k == 0),
                            stop=(k == KD - 1),
                        )
                    hm = pbig.tile(
                        [128, BC], dt.bfloat16, name=f"ht_{c}_{m}", tag=f"ht{m}"
                    )
                    nc.scalar.activation(
                        hm[:], ps[:], AF.Identity, bias=b1t[:, m : m + 1]
                    )
                    ht.append(hm)
                    if m == 6:
                        load_onehots()
                    if m == KH - 8:
                        load_w2(c, 0)
                    if m == KH - 4:
                        load_w2(c, 1)
                    if prev_tail[0] is not None and m in (1, 4):
                        # previous chunk's softmax tail halves: run on the PE
                        # here, long after its recip chain finished
                        prev_tail[0](half=0 if m == 1 else 1)
                        if m == 4:
                            prev_tail[0] = None

                # -------- phase B: yT = W2^T @ hT (+b2), e = exp(yT) -----
                # -------- + segment reduce / recip / broadcast / out -----
                et = [None] * KD
                spsum = {}
                r_tiles = {}
                groups_done = set()
                bcast_pending = list(range(KD))
                bcast_ready = []  # ready, emission delayed one B-group

                def emit_bcast(
                    m2, c=c, cs=cs, et=et, r_tiles=r_tiles, out_eng=None
                ):
                    gl = m_groups[m2]
                    pd = psum_d.tile(
                        [128, BC], dt.float32, name=f"pd_{c}_{m2}", tag="pd"
                    )
                    for idx, g in enumerate(gl):
                        nc.tensor.matmul(
                            pd[:],
                            ctt_all[:, kg_index[(m2, g)], :],
                            r_tiles[g][:],
                            start=(idx == 0),
                            stop=(idx == len(gl) - 1),
                        )
                    ot = pev.tile([128, BC], dt.bfloat16, name=f"ot_{c}_{m2}", tag="ot")
                    nc.vector.tensor_mul(ot[:], pd[:], et[m2][:])
                    (out_eng or nc.sync).dma_start(outp.ap()[m2][:, cs], ot[:])

                def flush_bcast(
                    max_n=None, bcast_ready=bcast_ready, emit_bcast=emit_bcast
                ):
                    n = len(bcast_ready) if max_n is None else max_n
                    for m2 in bcast_ready[:n]:
                        emit_bcast(m2)
                    del bcast_ready[:n]

                def emit_reduce(
                    k,
                    c=c,
                    et=et,
                    spsum=spsum,
                    r_tiles=r_tiles,
                    groups_done=groups_done,
                    bcast_pending=bcast_pending,
                    bcast_ready=bcast_ready,
                ):
                    for g in m_groups[k]:
                        if g not in spsum:
                            spsum[g] = psum_s.tile(
                                [128, BC], dt.float32, name=f"pss_{c}_{g}", tag="ps_s"
                            )
                        nc.tensor.matmul(
                            spsum[g][:],
                            cpt_all[:, kg_index[(k, g)], :],
                            et[k][:],
                            start=(k == k_first[g]),
                            stop=(k == k_last[g]),
                        )
                        if k == k_last[g]:
                            # reciprocal as exp(-ln(s)): two fast ACT passes
                            # (the DVE reciprocal takes 3.4us, 5x an ACT op),
                            # with the second pass narrowing to bf16 for the
                            # broadcast matmul. The empty segment row holds
                            # 2^-60 * e[feat0] (a normal float, never
                            # selected by any broadcast).
                            lg = pev.tile(
                                [128, BC], dt.float32, name=f"ls_{c}_{g}", tag="ls"
                            )
                            nc.scalar.activation(lg[:], spsum[g][:], AF.Ln)
                            rg = pbig.tile(
                                [128, BC], dt.bfloat16, name=f"r_{c}_{g}", tag=f"r{g}"
                            )
                            nc.scalar.activation(
                                rg[:], lg[:], AF.Exp, scale=-1.0
                            )
                            r_tiles[g] = rg
                            groups_done.add(g)
                            # queue feature tiles whose groups are all ready
                            still = []
                            for m2 in bcast_pending:
                                if et[m2] is not None and all(
                                    gg in groups_done for gg in m_groups[m2]
                                ):
                                    bcast_ready.append(m2)
                                else:
                                    still.append(m2)
                            bcast_pending[:] = still

                for m in range(KD):
                    if c + 1 < NB and m < KD // 2:
                        # trickle next chunk's x prefetch: one k-pair per
                        # B-group so it never bursts against the W2 stream
                        emit_xt_load(c + 1, pairs=[2 * m])
                    if m + 2 < KD and (c, m + 2) not in w2_tiles:
                        load_w2(c, m + 2)
                    if c + 1 < NB and KD - 5 <= m < KD - 1:
                        # next chunk's first w1 quad, triggered well ahead
                        load_w1(c + 1, m - (KD - 5))
                    w2t = w2_tiles.pop((c, m))
                    ps = psum_mm.tile(
                        [128, BC], dt.float32, name=f"psB_{c}_{m}", tag="mm"
                    )
                    for k in range(KH):
                        nc.tensor.matmul(
                            ps[:],
                            w2t[:, k, :],
                            ht[k][:],
                            start=(k == 0),
                            stop=(k == KH - 1),
                        )
                    em = pbig.tile(
                        [128, BC], dt.bfloat16, name=f"et_{c}_{m}", tag=f"et{m}"
                    )
                    nc.scalar.activation(em[:], ps[:], AF.Exp, bias=b2t[:, m : m + 1])
                    et[m] = em
                    # delayed work: bcasts queued >=1 B-group ago (trickled
                    # so DVE mult bursts never delay a reciprocal), then the
                    # reduce for k-tile m-1 (the lag hides ACT/DVE latency).
                    # At the last slot of the final chunk, drain everything
                    # ready — the muls overlap the remaining main matmuls.
                    flush_bcast(max_n=None if m == KD - 1 else 2)
                    if m >= 1:
                        emit_reduce(m - 1)

                # tail part 1 now: the final reduce + recip chain starts
                # immediately after the last B group
                emit_reduce(KD - 1)

                def tail(
                    half=None,
                    emit_bcast=emit_bcast,
                    bcast_pending=bcast_pending,
                    bcast_ready=bcast_ready,
                    final=(c + 1 == NB),
                ):
                    rest = bcast_ready + bcast_pending
                    if half == 0:
                        rest = rest[: (len(rest) + 1) // 2]
                    elif half == 1:
                        rest = rest[(len(rest) + 1) // 2 :]
                    for idx, m2 in enumerate(rest):
                        # in the final drain the ACT engine is idle, so
                        # alternate store triggers across both HWDGE ports;
                        # mid-kernel the ACT port is busy — keep sync only
                        emit_bcast(
                            m2,
                            out_eng=nc.scalar if (final and idx % 2) else nc.sync,
                        )
                    if half != 0:
                        del bcast_ready[:]
                        bcast_pending.clear()

                if c + 1 < NB:
                    # defer part 2: the PE executes the remaining broadcasts
                    # inside the next chunk's phase A, by which time the
                    # recip chain is long done
                    prev_tail[0] = tail
                else:
                    tail()

    _split_excess_waits(nc)
    return nc


def _pack_inputs(x, segment_ids, W1, b1, W2, b2):
    """Host-side shard + pack. Returns in_maps (one dict per core)."""
    import ml_dtypes

    bf16 = ml_dtypes.bfloat16
    seg = np.asarray(segment_ids).astype(np.int64)
    bounds, kg_pairs, k_first, _, _ = _segment_plan(seg)
    NKG = len(kg_pairs)
    barr = np.asarray(bounds)
    gof = np.searchsorted(barr, np.arange(S), side="right") - 1

    # one-hot tiles for the segment matmuls (partition-major packing)
    cp = np.zeros((NKG, 128, 128), dtype=bf16)
    ctp = np.zeros((NKG, 128, 128), dtype=bf16)
    for i, (k, g) in enumerate(kg_pairs):
        loc = seg[k * 128 : (k + 1) * 128] - barr[g]
        in_g = gof[seg[k * 128 : (k + 1) * 128]] == g
        rows = np.arange(128)
        cp[i, rows[in_g], loc[in_g]] = 1
        ctp[i, loc[in_g], rows[in_g]] = 1
    # any segment ROW with no contributing feature (empty real segments, and
    # the padding rows of groups smaller than 128) would sum to exactly 0 and
    # turn the Ln/Exp reciprocal into inf, which the broadcast matmul's 0.0
    # weights would then turn into NaN (0*inf). Plant a 2^-60 weight on
    # feature 0 of the group's first k-tile for those rows: their sums become
    # small-but-normal floats whose (finite, huge) reciprocals are multiplied
    # by exact 0.0 in every broadcast, contributing nothing.
    counts = np.bincount(seg, minlength=S)
    for g in range(len(bounds) - 1):
        i0 = kg_pairs.index((k_first[g], g))
        size_g = bounds[g + 1] - bounds[g]
        dead = [r for r in range(128) if r >= size_g or counts[barr[g] + r] == 0]
        for r in dead:
            cp[i0, 0, r] = 2.0**-60
    cpp = np.ascontiguousarray(cp.transpose(1, 0, 2))
    ctpp = np.ascontiguousarray(ctp.transpose(1, 0, 2))

    w1p = np.ascontiguousarray(
        W1.reshape(KD, 128, KH, 128).transpose(2, 1, 0, 3)
    ).astype(bf16)
    w2p = np.ascontiguousarray(
        W2.reshape(KH, 128, KD, 128).transpose(2, 1, 0, 3)
    ).astype(bf16)
    b1p = np.ascontiguousarray(b1.reshape(KH, 128).T).astype(np.float32)
    b2p = np.ascontiguousarray(b2.reshape(KD, 128).T).astype(np.float32)

    in_maps = []
    for core in range(NCORES):
        xs = x[core * BS : (core + 1) * BS]  # [BS, D]
        xtp = np.ascontiguousarray(
            xs.reshape(NB, BC, KD, 128).transpose(0, 3, 2, 1)
        ).astype(bf16)
        in_maps.append(
            {
                "xtp": xtp,
                "w1p": w1p,
                "w2p": w2p,
                "b1p": b1p,
                "b2p": b2p,
                "cpp": cpp,
                "ctpp": ctpp,
            }
        )
    return in_maps


def _unpack_outputs(results):
    """results: list (per core) of {"outp": [KD, 128, BS]} -> [B, D] f32."""
    parts = []
    for core in range(NCORES):
        outp = np.asarray(results[core]["outp"], dtype=np.float32)
        parts.append(np.ascontiguousarray(outp.transpose(2, 0, 1)).reshape(BS, D))
    return np.concatenate(parts, axis=0)


_CACHE = {}

# test harness hooks (not used in the graded path)
TRACE = False
TRACE_ALL_CORES = False
LAST_RESULT = None


def kernel(x, segment_ids, W1, b1, W2, b2):
    global LAST_RESULT
    _import_concourse()
    from concourse.bass_utils import run_bass_kernel_spmd

    key = np.asarray(segment_ids).tobytes()
    if key not in _CACHE:
        _CACHE[key] = _build_program(segment_ids)
    nc = _CACHE[key]

    in_maps = _pack_inputs(
        np.asarray(x, dtype=np.float32),
        segment_ids,
        np.asarray(W1, dtype=np.float32),
        np.asarray(b1, dtype=np.float32),
        np.asarray(W2, dtype=np.float32),
        np.asarray(b2, dtype=np.float32),
    )
    kw = {"trace_cores": list(range(NCORES))} if TRACE_ALL_CORES else {}
    res = run_bass_kernel_spmd(nc, in_maps, list(range(NCORES)), trace=TRACE, **kw)
    LAST_RESULT = res
    return _unpack_outputs(res.results)
